# revision 21
# baseline (speedup 1.0000x reference)
"""Multi-head attention Trainium2 kernel (8 NeuronCores, SPMD, no collectives).

Sharding: core c handles batch c//2, head-group c%2 (8 heads x 64 = 512 dims).
Wq/Wk/Wv column-sharded per head group, Wo row-sharded; the two partial
outputs per batch are summed on the host (plus the folded bv@Wo.T + bo bias).

bf16 matmul inputs / f32 PSUM accumulation. Structure:
  - scores = TWO CONCURRENT 64x128 row-tiled matmuls (T0 rows 0:64 = even
    head, T8 rows 64:128 = odd head),
  - DEEP scores->AV software pipeline (lag ~10 steps): AV matmuls always
    have their exp input long ready, so pass boundaries never stall the PE
    on the ACT engine catching up,
  - phase-1 work is woven between attention steps in <=2us blobs (the PSUM
    ring gives ~2 steps of score lookahead, so small blobs never starve
    the exp stream),
  - passes run qt-major; phase-3 chunks are split in half and hosted early
    in later passes, one half left for the drain tail,
  - softmax denominator rides as a ones-column in the AV stationary
    (VH is [.., 66] wide: 64 dk + ones + pad); its reciprocal is
    partition-broadcast by GPSIMD (no PE matmul).
"""

import numpy as np
import ml_dtypes
from contextlib import ExitStack

import concourse.bass as bass
import concourse.bacc as bacc
import concourse.mybir as mybir
import concourse.tile as tile
from concourse import library_config
from concourse.bass_utils import run_bass_kernel_spmd

B, S, D = 4, 2048, 1024
H, DK = 16, 64
NCORES = 8
HD = 512                  # head dims per group (8 heads x 64)
KC = D // 128             # 8 contraction chunks over d_model
NM = HD // 128            # 4 head pairs
NSCH = S // 128           # 16 S blocks of 128
VW = 66                   # VH stationary width: 64 dk + ones col + pad
AVLAG = 10                # scores->AV pipeline depth (steps)
F32 = mybir.dt.float32
BF16 = mybir.dt.bfloat16
FP = np.float32
BF = ml_dtypes.bfloat16


def build_core_program(nc, knobs=()):
    knobs = set(knobs)
    xqT = nc.declare_dram_parameter("xqT", [D, S], BF16, isOutput=False)
    xkT = nc.declare_dram_parameter("xkT", [D, S], BF16, isOutput=False)
    xvT = nc.declare_dram_parameter("xvT", [D, S], BF16, isOutput=False)
    wqT = nc.declare_dram_parameter("wqT", [D, HD], BF16, isOutput=False)
    wkT = nc.declare_dram_parameter("wkT", [D, HD], BF16, isOutput=False)
    wvT = nc.declare_dram_parameter("wvT", [D, HD], BF16, isOutput=False)
    woT = nc.declare_dram_parameter("woT", [HD, D], BF16, isOutput=False)
    bq = nc.declare_dram_parameter("bq", [128, NM], F32, isOutput=False)
    bk = nc.declare_dram_parameter("bk", [128, NM], F32, isOutput=False)
    out = nc.declare_dram_parameter("out", [S, D], F32, isOutput=True)

    with tile.TileContext(nc) as tc, ExitStack() as ctx:
        pBig = ctx.enter_context(tc.tile_pool(name="big", bufs=1))
        pWo = ctx.enter_context(tc.tile_pool(name="wo", bufs=1))
        pQKV = ctx.enter_context(tc.tile_pool(name="qkv", bufs=1))
        pX2 = ctx.enter_context(tc.tile_pool(name="x2", bufs=16))
        pXV = ctx.enter_context(tc.tile_pool(name="xv", bufs=12))
        pExp = ctx.enter_context(tc.tile_pool(name="exp", bufs=14))
        pSmall = ctx.enter_context(tc.tile_pool(name="small", bufs=1))
        pRec = ctx.enter_context(tc.tile_pool(name="rec", bufs=4))
        pBC = ctx.enter_context(tc.tile_pool(name="bc", bufs=4))
        pNrm = ctx.enter_context(tc.tile_pool(name="nrm", bufs=6))
        pOutF = ctx.enter_context(tc.tile_pool(name="outf", bufs=3))
        # PSUM: av accumulators (2 banks) + shared [128,1024] ring (6 banks)
        psA = ctx.enter_context(tc.tile_pool(name="ps_a", bufs=2, space="PSUM"))
        psS = ctx.enter_context(tc.tile_pool(name="ps_s", bufs=3, space="PSUM"))

        # ---- resident weights / biases ----
        qkvW = pBig.tile([128, 3, KC, HD], BF16, tag="qkvw")
        bqS = pSmall.tile([128, NM], F32, tag="bq")
        bkS = pSmall.tile([128, NM], F32, tag="bk")
        nc.sync.dma_start(bqS[:], bq[:])
        nc.sync.dma_start(bkS[:], bk[:])

        def load_w(i):
            w = (wqT, wkT, wvT)[i]
            for c in range(KC):
                nc.sync.dma_start(qkvW[:, i, c, :], w[c * 128:(c + 1) * 128, :])
        woS = pWo.tile([128, NM, D], BF16)

        def load_wo():
            for mc in range(NM):
                nc.sync.dma_start(woS[:, mc, :], woT[mc * 128:(mc + 1) * 128, :])

        # ---- resident activations ----
        QT = pQKV.tile([128, NM, S], BF16, tag="qt")      # [pair dims, S]
        KT = pQKV.tile([128, NM, S], BF16, tag="kt")      # packed pair khT
        # col 64 = ones (softmax denominator rides in the AV stationary)
        VH = pQKV.tile([128, NSCH, 8, VW], BF16, tag="vh")
        nc.vector.memset(VH[:], 0.0)
        nc.vector.memset(VH[:, :, :, 64:65], 1.0)
        outT = pBig.tile([128, NM, S], BF16, tag="outt")  # [pair dims, S]

        if 'fake_p1' in knobs:  # timing experiments: satisfy deps cheaply
            knobs.add('no_p1')
            nc.vector.memset(QT[:], 0.001)
            nc.vector.memset(KT[:], 0.001)
            nc.vector.memset(VH[:], 1.0)

        # ---- phase 1 emitters (all blobs <= ~2-4us) ----
        def emit_qk_dma(i, T):
            xT = (xqT, xkT)[i]
            xts = [pX2.tile([128, 2, 512], BF16, tag="x", name=f"x{i}{T}{_c}")
                   for _c in range(KC)]
            for c in range(KC):
                for h2 in range(2):
                    nc.sync.dma_start(
                        xts[c][:, h2, :],
                        xT[c * 128:(c + 1) * 128,
                           T * 1024 + h2 * 512:T * 1024 + (h2 + 1) * 512])
            return xts

        def emit_qk_mm(i, T, xts, m):
            dst, bias = ((QT, bqS), (KT, bkS))[i]
            acc = psS.tile([128, 1024], F32, tag="sc", name=f"qk{i}{T}{m}")
            for h2 in range(2):
                for c in range(KC):
                    nc.tensor.matmul(
                        acc[:, h2 * 512:(h2 + 1) * 512],
                        qkvW[:, i, c, m * 128:(m + 1) * 128],
                        xts[c][:, h2, :],
                        start=(c == 0), stop=(c == KC - 1))
            nc.vector.tensor_scalar_add(
                dst[:, m, T * 1024:(T + 1) * 1024], acc[:], bias[:, m:m + 1])

        def emit_v_dma(t):
            xts = [pXV.tile([128, 512], BF16, tag="xv", name=f"xv{t}{_c}")
                   for _c in range(KC)]
            for c in range(KC):
                nc.sync.dma_start(
                    xts[c][:], xvT[c * 128:(c + 1) * 128, t * 512:(t + 1) * 512])
            return xts

        def emit_v_q(t, xts, u01, j):
            """One quarter of a V tile: 8 matmuls -> VH sch block 4t+2u+j."""
            sch = t * 4 + u01 * 2 + j
            acc = psS.tile([128, 512], F32, tag="sc", name=f"v{sch}")
            for c in range(KC):
                nc.tensor.matmul(
                    acc[:],
                    xts[c][:, (u01 * 2 + j) * 128:(u01 * 2 + j + 1) * 128],
                    qkvW[:, 2, c, :],
                    start=(c == 0), stop=(c == KC - 1))
            nc.vector.tensor_copy(
                VH[:, sch, :, 0:64], acc[:].rearrange("p (h d) -> p h d", h=8))

        # ---- phase 2 state ----
        acc2 = {}           # live AV accumulators for the current pass
        pending = []        # deferred normalize tails
        p3ready = []        # (sch, nt) halves whose qt column is flushed
        step_no = [0]
        prevq = []          # deep scores->av pipeline

        def emit_scores_exp(mh, qt, kb):
            et = pExp.tile([128, 1024], BF16, tag="expt",
                           name=f"et{mh}_{qt}_{kb}")
            sp = psS.tile([128, 1024], F32, tag="sc", name=f"sp{mh}_{qt}_{kb}")
            # two concurrent 64x128 row tiles: T0 = even head, T8 = odd head
            nc.tensor.matmul(
                sp[:, 0:512],
                KT[0:64, mh, kb * 128:(kb + 1) * 128],
                QT[0:64, mh, qt * 512:(qt + 1) * 512],
                start=True, stop=True)
            nc.tensor.matmul(
                sp[:, 512:1024],
                KT[64:128, mh, kb * 128:(kb + 1) * 128],
                QT[64:128, mh, qt * 512:(qt + 1) * 512],
                start=True, stop=True)
            if 'no_exp' not in knobs:
                nc.scalar.activation(
                    et[:], sp[:],
                    mybir.ActivationFunctionType.Exp, scale=0.125)
            return et

        def emit_av(mh, qt, kb, et):
            if 'no_av' in knobs:
                return
            if kb == 0:
                acc2[(mh, qt)] = [
                    psA.tile([128, 512], F32, tag="acc", name=f"av{mh}_{qt}{_h}")
                    for _h in range(2)]
            for hh in range(2):
                nc.tensor.matmul(
                    acc2[(mh, qt)][hh][0:65, :], VH[:, kb, 2 * mh + hh, 0:65],
                    et[:, hh * 512:(hh + 1) * 512],
                    start=(kb == 0), stop=(kb == NSCH - 1))
            if kb == NSCH - 1 and 'no_norm' not in knobs:
                for hh in range(2):
                    # copy PSUM->SBUF fast so the accumulator bank frees
                    avs = pNrm.tile([65, 512], F32, tag="avs",
                                    name=f"avs{mh}_{qt}_{hh}")
                    nc.vector.tensor_copy(avs[:], acc2[(mh, qt)][hh][0:65, :])
                    recb = pRec.tile([1, 512], BF16, tag="recb",
                                     name=f"recb{mh}_{qt}_{hh}")
                    with nc.allow_low_precision("bf16 softmax reciprocal"):
                        nc.vector.reciprocal(recb[:], avs[64:65, :])
                    # due late enough that the 3.2us reciprocal has finished
                    pending.append((step_no[0] + (9 if hh == 0 else 11),
                                    hh * 64, mh, qt, avs, recb))
                del acc2[(mh, qt)]

        def flush_norm():
            # partition-broadcast 1/denom on GPSIMD, multiply, place in outT
            _, hp, mh, qt, avs, recb = pending.pop(0)
            bcb = pBC.tile([64, 512], BF16, tag="bcb", name=f"bc{mh}{qt}{hp}")
            nc.gpsimd.partition_broadcast(bcb[:], recb[:], channels=64)
            nrm = pNrm.tile([64, 512], BF16, tag="nrm", name=f"nrm{mh}{qt}{hp}")
            nc.vector.tensor_mul(nrm[:], avs[0:64, :], bcb[:])
            nc.sync.dma_start(
                outT[hp:hp + 64, mh, qt * 512:(qt + 1) * 512], nrm[:])
            if mh == NM - 1 and hp == 64:
                # last flush of this qt column: its p3 halves become ready
                p3ready.extend((qt * 4 + j, nt)
                               for j in range(4) for nt in range(2))

        def emit_se_step(mh, qt, kb):
            # ready work first (deferred AV, due flushes): the PE queue is
            # in-order, so it must never sit behind a blocked scores matmul
            while len(prevq) >= AVLAG:
                emit_av(*prevq.pop(0))
            step_no[0] += 1
            while pending and step_no[0] >= pending[0][0]:
                flush_norm()
            et = emit_scores_exp(mh, qt, kb)
            prevq.append((mh, qt, kb, et))

        # ---- phase 3 emitter: one 512-wide half (4 matmuls) at a time ----
        def emit_p3_half(sch, nt):
            fp = psS.tile([128, 512], F32, tag="sc", name=f"fp{sch}_{nt}")
            for mc in range(NM):
                nc.tensor.matmul(
                    fp[:], outT[:, mc, sch * 128:(sch + 1) * 128],
                    woS[:, mc, nt * 512:(nt + 1) * 512],
                    start=(mc == 0), stop=(mc == NM - 1))
            of = pOutF.tile([128, 512], F32, tag="of", name=f"of{sch}_{nt}")
            nc.vector.tensor_copy(of[:], fp[:])
            nc.sync.dma_start(
                out[sch * 128:(sch + 1) * 128, nt * 512:(nt + 1) * 512], of[:])

        def drain_rest():
            # tail: remaining AVs are ready work; weave reserved p3 halves in
            while prevq:
                emit_av(*prevq.pop(0))
                if p3ready and len(prevq) % 3 == 0 and 'no_p3' not in knobs:
                    emit_p3_half(*p3ready.pop(0))
            while pending:
                flush_norm()
            if 'no_p3' not in knobs:
                while p3ready:
                    emit_p3_half(*p3ready.pop(0))

        # ---- emission sequence ----
        weave = {}
        if 'no_p1' not in knobs:
            load_w(0)
            xq0 = emit_qk_dma(0, 0)
            load_w(1)
            xk0 = emit_qk_dma(1, 0)
            load_w(2)
            xv0 = emit_v_dma(0)
            emit_qk_mm(0, 0, xq0, 0)
            emit_qk_mm(1, 0, xk0, 0)
            st = {}  # mutable closure state for woven dmas
            # ordering constraints: ALL readers of an x-tile set must be
            # emitted before the dma that reuses its pool slots (pX2: xk1
            # reuses xq0's slots, xq1 reuses xk0's; pXV rotates similarly).
            # AVLAG relaxes VH gating: VH block j is needed at step j+AVLAG.
            weave = {
                (0, 0): [
                    (0, lambda: emit_qk_mm(0, 0, xq0, 1)),
                    (1, lambda: emit_qk_mm(1, 0, xk0, 1)),
                    (2, lambda: emit_qk_mm(0, 0, xq0, 2)),
                    (3, lambda: emit_qk_mm(1, 0, xk0, 2)),
                    (4, lambda: emit_qk_mm(0, 0, xq0, 3)),
                    (5, lambda: emit_qk_mm(1, 0, xk0, 3)),
                    (5, lambda: st.__setitem__('xk1', emit_qk_dma(1, 1))),
                    (6, lambda: emit_v_q(0, xv0, 0, 0)),
                    (7, lambda: emit_qk_mm(1, 1, st['xk1'], 0)),
                    (7, lambda: emit_v_q(0, xv0, 0, 1)),
                    (8, lambda: emit_v_q(0, xv0, 1, 0)),
                    (9, lambda: emit_v_q(0, xv0, 1, 1)),
                    (10, lambda: st.__setitem__('xv1', emit_v_dma(1))),
                    (11, lambda: emit_v_q(1, st['xv1'], 0, 0)),
                    (12, lambda: emit_v_q(1, st['xv1'], 0, 1)),
                    (13, lambda: emit_v_q(1, st['xv1'], 1, 0)),
                    (14, lambda: emit_v_q(1, st['xv1'], 1, 1)),
                    (15, lambda: st.__setitem__('xv2', emit_v_dma(2))),
                ],
                (1, 0): [
                    (0, lambda: emit_v_q(2, st['xv2'], 0, 0)),
                    (1, lambda: emit_v_q(2, st['xv2'], 0, 1)),
                    (2, lambda: emit_v_q(2, st['xv2'], 1, 0)),
                    (3, lambda: emit_v_q(2, st['xv2'], 1, 1)),
                    (4, lambda: st.__setitem__('xv3', emit_v_dma(3))),
                    (4, lambda: emit_v_q(3, st['xv3'], 0, 0)),
                    (5, lambda: emit_v_q(3, st['xv3'], 0, 1)),
                    (6, lambda: emit_v_q(3, st['xv3'], 1, 0)),
                    (7, lambda: emit_v_q(3, st['xv3'], 1, 1)),
                    (7, lambda: emit_qk_mm(1, 1, st['xk1'], 1)),
                    (10, load_wo),
                ],
                (2, 0): [
                    (0, lambda: st.__setitem__('xq1', emit_qk_dma(0, 1))),
                    (2, lambda: emit_qk_mm(1, 1, st['xk1'], 2)),
                    (4, lambda: emit_qk_mm(0, 1, st['xq1'], 0)),
                    (6, lambda: emit_qk_mm(0, 1, st['xq1'], 1)),
                    (8, lambda: emit_qk_mm(0, 1, st['xq1'], 2)),
                    (10, lambda: emit_qk_mm(0, 1, st['xq1'], 3)),
                ],
                (3, 0): [
                    (2, lambda: emit_qk_mm(1, 1, st['xk1'], 3)),
                ],
            }
        elif 'no_p2' not in knobs:
            load_wo()

        if 'no_p2' not in knobs:
            for qt in range(4):
                for mh in range(NM):
                    todo = sorted(weave.get((mh, qt), []), key=lambda x: x[0])
                    for kb in range(NSCH):
                        while todo and todo[0][0] <= kb:
                            todo.pop(0)[1]()
                        # host ready p3 halves away from pass boundaries;
                        # none in the last pass (reserved for the tail)
                        if (kb in (2, 5, 8, 11) and p3ready
                                and 'no_p3' not in knobs):
                            emit_p3_half(*p3ready.pop(0))
                        emit_se_step(mh, qt, kb)
            drain_rest()
        elif 'no_p1' not in knobs:
            # projections only (timing knob)
            emit_qk_mm(0, 0, xq0, 1)
            emit_qk_mm(0, 0, xq0, 2)
            emit_qk_mm(0, 0, xq0, 3)
            emit_qk_mm(1, 0, xk0, 1)
            emit_qk_mm(1, 0, xk0, 2)
            emit_qk_mm(1, 0, xk0, 3)
            for u01 in range(2):
                for j in range(2):
                    emit_v_q(0, xv0, u01, j)
            xq1 = emit_qk_dma(0, 1)
            xk1 = emit_qk_dma(1, 1)
            for m in range(NM):
                emit_qk_mm(0, 1, xq1, m)
                emit_qk_mm(1, 1, xk1, m)
            for t in range(1, 4):
                xv = emit_v_dma(t)
                for u01 in range(2):
                    for j in range(2):
                        emit_v_q(t, xv, u01, j)
            load_wo()
    return nc


def make_in_maps(q, k, v, Wq, bq, Wk, bk, Wv, bv, Wo, bo):
    """Shard + pre-transpose the full inputs into the 8 per-core maps."""
    q, k, v = (np.asarray(t, FP) for t in (q, k, v))
    Wq, bq, Wk, bk = (np.asarray(t, FP) for t in (Wq, bq, Wk, bk))
    Wv, bv, Wo, bo = (np.asarray(t, FP) for t in (Wv, bv, Wo, bo))
    maps = []
    for c in range(NCORES):
        b, g = c // 2, c % 2
        sl = slice(g * HD, (g + 1) * HD)
        maps.append({
            "xqT": np.ascontiguousarray(q[b].T).astype(BF),
            "xkT": np.ascontiguousarray(k[b].T).astype(BF),
            "xvT": np.ascontiguousarray(v[b].T).astype(BF),
            "wqT": np.ascontiguousarray(Wq[sl, :].T).astype(BF),
            "wkT": np.ascontiguousarray(Wk[sl, :].T).astype(BF),
            "wvT": np.ascontiguousarray(Wv[sl, :].T).astype(BF),
            "woT": np.ascontiguousarray(Wo[:, sl].T).astype(BF),
            "bq": np.ascontiguousarray(bq[sl].reshape(NM, 128).T),
            "bk": np.ascontiguousarray(bk[sl].reshape(NM, 128).T),
        })
    return maps


_CACHE = {}


def _get_program():
    if "nc" not in _CACHE:
        nc = bacc.Bacc("TRN2", target_bir_lowering=False, debug=False)
        build_core_program(nc)
        nc.compile()
        _CACHE["nc"] = nc
    return _CACHE["nc"]


def run(inputs, trace=False, **kw):
    """Run on the 8 NeuronCores; returns (full_output, BassKernelResults)."""
    nc = _get_program()
    in_maps = make_in_maps(**inputs)
    res = run_bass_kernel_spmd(
        nc, in_maps, core_ids=list(range(NCORES)), trace=trace, **kw)
    bv = np.asarray(inputs["bv"], FP)
    Wo = np.asarray(inputs["Wo"], FP)
    bo = np.asarray(inputs["bo"], FP)
    bias = bo + bv @ Wo.T
    full = np.empty((B, S, D), FP)
    for b in range(B):
        full[b] = (res.results[2 * b]["out"] + res.results[2 * b + 1]["out"]
                   + bias)
    return full, res


def kernel(**inputs) -> np.ndarray:
    # mask is all-ones by construction (spec fill: "ones") -> identity
    inputs.pop("mask", None)
    out, _ = run(inputs)
    return out


# revision 24
# speedup vs baseline: 1.0214x; 1.0214x over previous
"""Multi-head attention Trainium2 kernel (8 NeuronCores, SPMD, no collectives).

Sharding: core c handles batch c//2, head-group c%2 (8 heads x 64 = 512 dims).
Wq/Wk/Wv column-sharded per head group, Wo row-sharded; the two partial
outputs per batch are summed on the host (plus the folded bv@Wo.T + bo bias).

bf16 matmul inputs / f32 PSUM accumulation. Structure:
  - scores = TWO CONCURRENT 64x128 row-tiled matmuls (T0 rows 0:64 = even
    head, T8 rows 64:128 = odd head),
  - DEEP scores->AV software pipeline (lag ~10 steps): AV matmuls always
    have their exp input long ready, so pass boundaries never stall the PE
    on the ACT engine catching up,
  - phase-1 work is woven between attention steps in <=2us blobs (the PSUM
    ring gives ~2 steps of score lookahead, so small blobs never starve
    the exp stream),
  - passes run qt-major; phase-3 chunks are split in half and hosted early
    in later passes, one half left for the drain tail,
  - softmax denominator rides as a ones-column in the AV stationary
    (VH is [.., 66] wide: 64 dk + ones + pad); its reciprocal is
    partition-broadcast by GPSIMD (no PE matmul).
"""

import numpy as np
import ml_dtypes
from contextlib import ExitStack

import concourse.bass as bass
import concourse.bacc as bacc
import concourse.mybir as mybir
import concourse.tile as tile
from concourse import library_config
from concourse.bass_utils import run_bass_kernel_spmd

B, S, D = 4, 2048, 1024
H, DK = 16, 64
NCORES = 8
HD = 512                  # head dims per group (8 heads x 64)
KC = D // 128             # 8 contraction chunks over d_model
NM = HD // 128            # 4 head pairs
NSCH = S // 128           # 16 S blocks of 128
VW = 128                  # VH stationary width (full 128: FWL needs NumWeights==128)
AVLAG = 9                 # scores->AV pipeline depth (steps)
F32 = mybir.dt.float32
BF16 = mybir.dt.bfloat16
FP = np.float32
BF = ml_dtypes.bfloat16


def build_core_program(nc, knobs=()):
    knobs = set(knobs)
    xqT = nc.declare_dram_parameter("xqT", [D, S], BF16, isOutput=False)
    xkT = nc.declare_dram_parameter("xkT", [D, S], BF16, isOutput=False)
    xvT = nc.declare_dram_parameter("xvT", [D, S], BF16, isOutput=False)
    wqT = nc.declare_dram_parameter("wqT", [D, HD], BF16, isOutput=False)
    wkT = nc.declare_dram_parameter("wkT", [D, HD], BF16, isOutput=False)
    wvT = nc.declare_dram_parameter("wvT", [D, HD], BF16, isOutput=False)
    woT = nc.declare_dram_parameter("woT", [HD, D], BF16, isOutput=False)
    bq = nc.declare_dram_parameter("bq", [128, NM], F32, isOutput=False)
    bk = nc.declare_dram_parameter("bk", [128, NM], F32, isOutput=False)
    out = nc.declare_dram_parameter("out", [S, D], F32, isOutput=True)

    with tile.TileContext(nc) as tc, ExitStack() as ctx:
        pBig = ctx.enter_context(tc.tile_pool(name="big", bufs=1))
        pWo = ctx.enter_context(tc.tile_pool(name="wo", bufs=1))
        pQKV = ctx.enter_context(tc.tile_pool(name="qkv", bufs=1))
        pX2 = ctx.enter_context(tc.tile_pool(name="x2", bufs=16))
        pXV = ctx.enter_context(tc.tile_pool(name="xv", bufs=16))
        pExp = ctx.enter_context(tc.tile_pool(name="exp", bufs=11))
        pSmall = ctx.enter_context(tc.tile_pool(name="small", bufs=1))
        pRec = ctx.enter_context(tc.tile_pool(name="rec", bufs=3))
        pBC = ctx.enter_context(tc.tile_pool(name="bc", bufs=3))
        pNrm = ctx.enter_context(tc.tile_pool(name="nrm", bufs=5))
        pOutF = ctx.enter_context(tc.tile_pool(name="outf", bufs=2))
        # PSUM: av accumulators (2 banks) + shared [128,1024] ring (6 banks)
        psA = ctx.enter_context(tc.tile_pool(name="ps_a", bufs=2, space="PSUM"))
        psS = ctx.enter_context(tc.tile_pool(name="ps_s", bufs=3, space="PSUM"))

        # ---- resident weights / biases ----
        qkvW = pBig.tile([128, 3, KC, HD], BF16, tag="qkvw")
        bqS = pSmall.tile([128, NM], F32, tag="bq")
        bkS = pSmall.tile([128, NM], F32, tag="bk")
        nc.sync.dma_start(bqS[:], bq[:])
        nc.sync.dma_start(bkS[:], bk[:])

        def load_w(i):
            w = (wqT, wkT, wvT)[i]
            for c in range(KC):
                nc.sync.dma_start(qkvW[:, i, c, :], w[c * 128:(c + 1) * 128, :])
        woS = pWo.tile([128, NM, D], BF16)

        def load_wo():
            for mc in range(NM):
                nc.sync.dma_start(woS[:, mc, :], woT[mc * 128:(mc + 1) * 128, :])

        # ---- resident activations ----
        QT = pQKV.tile([128, NM, S], BF16, tag="qt")      # [pair dims, S]
        KT = pQKV.tile([128, NM, S], BF16, tag="kt")      # packed pair khT
        # col 64 = ones (softmax denominator rides in the AV stationary)
        VH = pQKV.tile([128, NSCH, 8, VW], BF16, tag="vh")
        nc.vector.memset(VH[:], 0.0)
        nc.vector.memset(VH[:, :, :, 64:65], 1.0)
        outT = pBig.tile([128, NM, S], BF16, tag="outt")  # [pair dims, S]

        if 'fake_p1' in knobs:  # timing experiments: satisfy deps cheaply
            knobs.add('no_p1')
            nc.vector.memset(QT[:], 0.001)
            nc.vector.memset(KT[:], 0.001)
            nc.vector.memset(VH[:], 1.0)

        # ---- phase 1 emitters (all blobs <= ~2-4us) ----
        def emit_qk_dma(i, T):
            xT = (xqT, xkT)[i]
            xts = [pX2.tile([128, 2, 512], BF16, tag="x", name=f"x{i}{T}{_c}")
                   for _c in range(KC)]
            for c in range(KC):
                for h2 in range(2):
                    nc.sync.dma_start(
                        xts[c][:, h2, :],
                        xT[c * 128:(c + 1) * 128,
                           T * 1024 + h2 * 512:T * 1024 + (h2 + 1) * 512])
            return xts

        def emit_qk_mm(i, T, xts, m):
            dst, bias = ((QT, bqS), (KT, bkS))[i]
            acc = psS.tile([128, 1024], F32, tag="sc", name=f"qk{i}{T}{m}")
            for h2 in range(2):
                for c in range(KC):
                    nc.tensor.matmul(
                        acc[:, h2 * 512:(h2 + 1) * 512],
                        qkvW[:, i, c, m * 128:(m + 1) * 128],
                        xts[c][:, h2, :],
                        start=(c == 0), stop=(c == KC - 1))
            nc.vector.tensor_scalar_add(
                dst[:, m, T * 1024:(T + 1) * 1024], acc[:], bias[:, m:m + 1])

        def emit_v_dma(t):
            xts = [pXV.tile([128, 512], BF16, tag="xv", name=f"xv{t}{_c}")
                   for _c in range(KC)]
            for c in range(KC):
                nc.sync.dma_start(
                    xts[c][:], xvT[c * 128:(c + 1) * 128, t * 512:(t + 1) * 512])
            return xts

        def emit_v_q(t, xts, u01, j):
            """One quarter of a V tile: 8 matmuls -> VH sch block 4t+2u+j."""
            sch = t * 4 + u01 * 2 + j
            acc = psS.tile([128, 512], F32, tag="sc", name=f"v{sch}")
            for c in range(KC):
                nc.tensor.matmul(
                    acc[:],
                    xts[c][:, (u01 * 2 + j) * 128:(u01 * 2 + j + 1) * 128],
                    qkvW[:, 2, c, :],
                    start=(c == 0), stop=(c == KC - 1))
            nc.vector.tensor_copy(
                VH[:, sch, :, 0:64], acc[:].rearrange("p (h d) -> p h d", h=8))

        # ---- phase 2 state ----
        acc2 = {}           # live AV accumulators for the current pass
        pending = []        # deferred normalize tails
        p3ready = []        # (sch, nt) halves whose qt column is flushed
        step_no = [0]
        prevq = []          # deep scores->av pipeline

        def emit_scores_exp(mh, qt, kb):
            et = pExp.tile([128, 1024], BF16, tag="expt",
                           name=f"et{mh}_{qt}_{kb}")
            sp = psS.tile([128, 1024], F32, tag="sc", name=f"sp{mh}_{qt}_{kb}")
            # two concurrent 64x128 row tiles: T0 = even head, T8 = odd head
            nc.tensor.matmul(
                sp[:, 0:512],
                KT[0:64, mh, kb * 128:(kb + 1) * 128],
                QT[0:64, mh, qt * 512:(qt + 1) * 512],
                start=True, stop=True)
            nc.tensor.matmul(
                sp[:, 512:1024],
                KT[64:128, mh, kb * 128:(kb + 1) * 128],
                QT[64:128, mh, qt * 512:(qt + 1) * 512],
                start=True, stop=True)
            if 'no_exp' not in knobs:
                nc.scalar.activation(
                    et[:], sp[:],
                    mybir.ActivationFunctionType.Exp, scale=0.125)
            return et

        def emit_av(mh, qt, kb, et):
            if 'no_av' in knobs:
                return
            if kb == 0:
                acc2[(mh, qt)] = [
                    psA.tile([128, 512], F32, tag="acc", name=f"av{mh}_{qt}{_h}")
                    for _h in range(2)]
            for hh in range(2):
                nc.tensor.matmul(
                    acc2[(mh, qt)][hh][:], VH[:, kb, 2 * mh + hh, :],
                    et[:, hh * 512:(hh + 1) * 512],
                    start=(kb == 0), stop=(kb == NSCH - 1))
            if kb == NSCH - 1 and 'no_norm' not in knobs:
                for hh in range(2):
                    # copy PSUM->SBUF fast so the accumulator bank frees
                    avs = pNrm.tile([65, 512], F32, tag="avs",
                                    name=f"avs{mh}_{qt}_{hh}")
                    nc.vector.tensor_copy(avs[:], acc2[(mh, qt)][hh][0:65, :])
                    recb = pRec.tile([1, 512], BF16, tag="recb",
                                     name=f"recb{mh}_{qt}_{hh}")
                    with nc.allow_low_precision("bf16 softmax reciprocal"):
                        nc.vector.reciprocal(recb[:], avs[64:65, :])
                    # due late enough that the 3.2us reciprocal has finished
                    pending.append((step_no[0] + (9 if hh == 0 else 11),
                                    hh * 64, mh, qt, avs, recb))
                del acc2[(mh, qt)]

        def flush_norm():
            # partition-broadcast 1/denom on GPSIMD, multiply, place in outT
            _, hp, mh, qt, avs, recb = pending.pop(0)
            bcb = pBC.tile([64, 512], BF16, tag="bcb", name=f"bc{mh}{qt}{hp}")
            nc.gpsimd.partition_broadcast(bcb[:], recb[:], channels=64)
            nrm = pNrm.tile([64, 512], BF16, tag="nrm", name=f"nrm{mh}{qt}{hp}")
            nc.vector.tensor_mul(nrm[:], avs[0:64, :], bcb[:])
            nc.sync.dma_start(
                outT[hp:hp + 64, mh, qt * 512:(qt + 1) * 512], nrm[:])
            if mh == NM - 1 and hp == 64:
                # last flush of this qt column: its p3 halves become ready
                p3ready.extend((qt * 4 + j, nt)
                               for j in range(4) for nt in range(2))

        def emit_se_step(mh, qt, kb):
            # ready work first (deferred AV, due flushes): the PE queue is
            # in-order, so it must never sit behind a blocked scores matmul
            while len(prevq) >= AVLAG:
                emit_av(*prevq.pop(0))
            step_no[0] += 1
            while pending and step_no[0] >= pending[0][0]:
                flush_norm()
            et = emit_scores_exp(mh, qt, kb)
            prevq.append((mh, qt, kb, et))

        # ---- phase 3 emitter: one 512-wide half (4 matmuls) at a time ----
        def emit_p3_half(sch, nt):
            fp = psS.tile([128, 512], F32, tag="sc", name=f"fp{sch}_{nt}")
            for mc in range(NM):
                nc.tensor.matmul(
                    fp[:], outT[:, mc, sch * 128:(sch + 1) * 128],
                    woS[:, mc, nt * 512:(nt + 1) * 512],
                    start=(mc == 0), stop=(mc == NM - 1))
            of = pOutF.tile([128, 512], F32, tag="of", name=f"of{sch}_{nt}")
            nc.vector.tensor_copy(of[:], fp[:])
            nc.sync.dma_start(
                out[sch * 128:(sch + 1) * 128, nt * 512:(nt + 1) * 512], of[:])

        def drain_rest():
            # tail: remaining AVs are ready work; weave reserved p3 halves in
            while prevq:
                emit_av(*prevq.pop(0))
                if p3ready and len(prevq) % 3 == 0 and 'no_p3' not in knobs:
                    emit_p3_half(*p3ready.pop(0))
            while pending:
                flush_norm()
            if 'no_p3' not in knobs:
                while p3ready:
                    emit_p3_half(*p3ready.pop(0))

        # ---- emission sequence ----
        weave = {}
        if 'no_p1' not in knobs:
            load_w(0)
            xq0 = emit_qk_dma(0, 0)
            load_w(1)
            xk0 = emit_qk_dma(1, 0)
            load_w(2)
            xv0 = emit_v_dma(0)
            emit_qk_mm(0, 0, xq0, 0)
            emit_qk_mm(1, 0, xk0, 0)
            st = {}  # mutable closure state for woven dmas
            # ordering constraints: ALL readers of an x-tile set must be
            # emitted before the dma that reuses its pool slots (pX2: xk1
            # reuses xq0's slots, xq1 reuses xk0's; pXV rotates similarly).
            # AVLAG relaxes VH gating: VH block j is needed at step j+AVLAG.
            weave = {
                (0, 0): [
                    (0, lambda: emit_qk_mm(0, 0, xq0, 1)),
                    (1, lambda: emit_qk_mm(1, 0, xk0, 1)),
                    (2, lambda: st.__setitem__('xv1', emit_v_dma(1))),
                    (2, lambda: emit_qk_mm(0, 0, xq0, 2)),
                    (3, lambda: emit_qk_mm(1, 0, xk0, 2)),
                    (4, lambda: emit_qk_mm(0, 0, xq0, 3)),
                    (5, lambda: emit_qk_mm(1, 0, xk0, 3)),
                    (5, lambda: st.__setitem__('xk1', emit_qk_dma(1, 1))),
                    (6, lambda: emit_v_q(0, xv0, 0, 0)),
                    (7, lambda: emit_v_q(0, xv0, 0, 1)),
                    (8, lambda: emit_qk_mm(1, 1, st['xk1'], 0)),
                    (9, lambda: emit_v_q(0, xv0, 1, 0)),
                    (10, lambda: emit_v_q(0, xv0, 1, 1)),
                    (11, lambda: st.__setitem__('xv2', emit_v_dma(2))),
                    (12, lambda: emit_v_q(1, st['xv1'], 0, 0)),
                    (13, lambda: emit_v_q(1, st['xv1'], 0, 1)),
                    (14, lambda: emit_v_q(1, st['xv1'], 1, 0)),
                    (15, lambda: emit_v_q(1, st['xv1'], 1, 1)),
                ],
                (1, 0): [
                    (0, lambda: st.__setitem__('xv3', emit_v_dma(3))),
                    (0, lambda: emit_v_q(2, st['xv2'], 0, 0)),
                    (1, lambda: emit_v_q(2, st['xv2'], 0, 1)),
                    (2, lambda: emit_v_q(2, st['xv2'], 1, 0)),
                    (3, lambda: emit_v_q(2, st['xv2'], 1, 1)),
                    (4, lambda: emit_v_q(3, st['xv3'], 0, 0)),
                    (5, lambda: emit_v_q(3, st['xv3'], 0, 1)),
                    (6, lambda: emit_v_q(3, st['xv3'], 1, 0)),
                    (7, lambda: emit_v_q(3, st['xv3'], 1, 1)),
                    (8, lambda: emit_qk_mm(1, 1, st['xk1'], 1)),
                    (11, load_wo),
                ],
                (2, 0): [
                    (0, lambda: st.__setitem__('xq1', emit_qk_dma(0, 1))),
                    (2, lambda: emit_qk_mm(1, 1, st['xk1'], 2)),
                    (4, lambda: emit_qk_mm(0, 1, st['xq1'], 0)),
                    (6, lambda: emit_qk_mm(0, 1, st['xq1'], 1)),
                    (8, lambda: emit_qk_mm(0, 1, st['xq1'], 2)),
                    (10, lambda: emit_qk_mm(0, 1, st['xq1'], 3)),
                ],
                (3, 0): [
                    (2, lambda: emit_qk_mm(1, 1, st['xk1'], 3)),
                ],
            }
        elif 'no_p2' not in knobs:
            load_wo()

        if 'no_p2' not in knobs:
            for qt in range(4):
                for mh in range(NM):
                    todo = sorted(weave.get((mh, qt), []), key=lambda x: x[0])
                    for kb in range(NSCH):
                        while todo and todo[0][0] <= kb:
                            todo.pop(0)[1]()
                        # host ready p3 halves away from pass boundaries;
                        # none in the last pass (reserved for the tail)
                        if (kb in (2, 5, 8, 11) and p3ready
                                and 'no_p3' not in knobs):
                            emit_p3_half(*p3ready.pop(0))
                        emit_se_step(mh, qt, kb)
            drain_rest()
        elif 'no_p1' not in knobs:
            # projections only (timing knob)
            emit_qk_mm(0, 0, xq0, 1)
            emit_qk_mm(0, 0, xq0, 2)
            emit_qk_mm(0, 0, xq0, 3)
            emit_qk_mm(1, 0, xk0, 1)
            emit_qk_mm(1, 0, xk0, 2)
            emit_qk_mm(1, 0, xk0, 3)
            for u01 in range(2):
                for j in range(2):
                    emit_v_q(0, xv0, u01, j)
            xq1 = emit_qk_dma(0, 1)
            xk1 = emit_qk_dma(1, 1)
            for m in range(NM):
                emit_qk_mm(0, 1, xq1, m)
                emit_qk_mm(1, 1, xk1, m)
            for t in range(1, 4):
                xv = emit_v_dma(t)
                for u01 in range(2):
                    for j in range(2):
                        emit_v_q(t, xv, u01, j)
            load_wo()
    return nc


def make_in_maps(q, k, v, Wq, bq, Wk, bk, Wv, bv, Wo, bo):
    """Shard + pre-transpose the full inputs into the 8 per-core maps."""
    q, k, v = (np.asarray(t, FP) for t in (q, k, v))
    Wq, bq, Wk, bk = (np.asarray(t, FP) for t in (Wq, bq, Wk, bk))
    Wv, bv, Wo, bo = (np.asarray(t, FP) for t in (Wv, bv, Wo, bo))
    maps = []
    for c in range(NCORES):
        b, g = c // 2, c % 2
        sl = slice(g * HD, (g + 1) * HD)
        maps.append({
            "xqT": np.ascontiguousarray(q[b].T).astype(BF),
            "xkT": np.ascontiguousarray(k[b].T).astype(BF),
            "xvT": np.ascontiguousarray(v[b].T).astype(BF),
            "wqT": np.ascontiguousarray(Wq[sl, :].T).astype(BF),
            "wkT": np.ascontiguousarray(Wk[sl, :].T).astype(BF),
            "wvT": np.ascontiguousarray(Wv[sl, :].T).astype(BF),
            "woT": np.ascontiguousarray(Wo[:, sl].T).astype(BF),
            "bq": np.ascontiguousarray(bq[sl].reshape(NM, 128).T),
            "bk": np.ascontiguousarray(bk[sl].reshape(NM, 128).T),
        })
    return maps


_CACHE = {}


def _get_program():
    if "nc" not in _CACHE:
        nc = bacc.Bacc("TRN2", target_bir_lowering=False, debug=False)
        build_core_program(nc)
        nc.compile()
        _CACHE["nc"] = nc
    return _CACHE["nc"]


def run(inputs, trace=False, **kw):
    """Run on the 8 NeuronCores; returns (full_output, BassKernelResults)."""
    nc = _get_program()
    in_maps = make_in_maps(**inputs)
    res = run_bass_kernel_spmd(
        nc, in_maps, core_ids=list(range(NCORES)), trace=trace, **kw)
    bv = np.asarray(inputs["bv"], FP)
    Wo = np.asarray(inputs["Wo"], FP)
    bo = np.asarray(inputs["bo"], FP)
    bias = bo + bv @ Wo.T
    full = np.empty((B, S, D), FP)
    for b in range(B):
        full[b] = (res.results[2 * b]["out"] + res.results[2 * b + 1]["out"]
                   + bias)
    return full, res


def kernel(**inputs) -> np.ndarray:
    # mask is all-ones by construction (spec fill: "ones") -> identity
    inputs.pop("mask", None)
    out, _ = run(inputs)
    return out


# revision 25
# speedup vs baseline: 1.0218x; 1.0004x over previous
"""Multi-head attention Trainium2 kernel (8 NeuronCores, SPMD, no collectives).

Sharding: core c handles batch c//2, head-group c%2 (8 heads x 64 = 512 dims).
Wq/Wk/Wv column-sharded per head group, Wo row-sharded; the two partial
outputs per batch are summed on the host (plus the folded bv@Wo.T + bo bias).

bf16 matmul inputs / f32 PSUM accumulation. Structure:
  - scores = TWO CONCURRENT 64x128 row-tiled matmuls (T0 rows 0:64 = even
    head, T8 rows 64:128 = odd head),
  - DEEP scores->AV software pipeline (lag ~10 steps): AV matmuls always
    have their exp input long ready, so pass boundaries never stall the PE
    on the ACT engine catching up,
  - phase-1 work is woven between attention steps in <=2us blobs (the PSUM
    ring gives ~2 steps of score lookahead, so small blobs never starve
    the exp stream),
  - passes run qt-major; phase-3 chunks are split in half and hosted early
    in later passes, one half left for the drain tail,
  - softmax denominator rides as a ones-column in the AV stationary
    (VH is [.., 66] wide: 64 dk + ones + pad); its reciprocal is
    partition-broadcast by GPSIMD (no PE matmul).
"""

import numpy as np
import ml_dtypes
from contextlib import ExitStack

import concourse.bass as bass
import concourse.bacc as bacc
import concourse.mybir as mybir
import concourse.tile as tile
from concourse import library_config
from concourse.bass_utils import run_bass_kernel_spmd

B, S, D = 4, 2048, 1024
H, DK = 16, 64
NCORES = 8
HD = 512                  # head dims per group (8 heads x 64)
KC = D // 128             # 8 contraction chunks over d_model
NM = HD // 128            # 4 head pairs
NSCH = S // 128           # 16 S blocks of 128
VW = 128                  # VH stationary width (full 128: FWL needs NumWeights==128)
AVLAG = 9                 # scores->AV pipeline depth (steps)
F32 = mybir.dt.float32
BF16 = mybir.dt.bfloat16
FP = np.float32
BF = ml_dtypes.bfloat16


def build_core_program(nc, knobs=()):
    knobs = set(knobs)
    xqT = nc.declare_dram_parameter("xqT", [D, S], BF16, isOutput=False)
    xkT = nc.declare_dram_parameter("xkT", [D, S], BF16, isOutput=False)
    xvT = nc.declare_dram_parameter("xvT", [D, S], BF16, isOutput=False)
    wqT = nc.declare_dram_parameter("wqT", [D, HD], BF16, isOutput=False)
    wkT = nc.declare_dram_parameter("wkT", [D, HD], BF16, isOutput=False)
    wvT = nc.declare_dram_parameter("wvT", [D, HD], BF16, isOutput=False)
    woT = nc.declare_dram_parameter("woT", [HD, D], BF16, isOutput=False)
    bq = nc.declare_dram_parameter("bq", [128, NM], F32, isOutput=False)
    bk = nc.declare_dram_parameter("bk", [128, NM], F32, isOutput=False)
    out = nc.declare_dram_parameter("out", [S, D], F32, isOutput=True)

    with tile.TileContext(nc) as tc, ExitStack() as ctx:
        pBig = ctx.enter_context(tc.tile_pool(name="big", bufs=1))
        pWo = ctx.enter_context(tc.tile_pool(name="wo", bufs=1))
        pQKV = ctx.enter_context(tc.tile_pool(name="qkv", bufs=1))
        pX2 = ctx.enter_context(tc.tile_pool(name="x2", bufs=16))
        pXV = ctx.enter_context(tc.tile_pool(name="xv", bufs=16))
        pExp = ctx.enter_context(tc.tile_pool(name="exp", bufs=11))
        pSmall = ctx.enter_context(tc.tile_pool(name="small", bufs=1))
        pRec = ctx.enter_context(tc.tile_pool(name="rec", bufs=3))
        pBC = ctx.enter_context(tc.tile_pool(name="bc", bufs=3))
        pNrm = ctx.enter_context(tc.tile_pool(name="nrm", bufs=5))
        pOutF = ctx.enter_context(tc.tile_pool(name="outf", bufs=2))
        # PSUM: av accumulators (2 banks) + shared [128,1024] ring (6 banks)
        psA = ctx.enter_context(tc.tile_pool(name="ps_a", bufs=2, space="PSUM"))
        psS = ctx.enter_context(tc.tile_pool(name="ps_s", bufs=3, space="PSUM"))

        # ---- resident weights / biases ----
        qkvW = pBig.tile([128, 3, KC, HD], BF16, tag="qkvw")
        bqS = pSmall.tile([128, NM], F32, tag="bq")
        bkS = pSmall.tile([128, NM], F32, tag="bk")
        nc.sync.dma_start(bqS[:], bq[:])
        nc.sync.dma_start(bkS[:], bk[:])

        def load_w(i):
            w = (wqT, wkT, wvT)[i]
            for c in range(KC):
                nc.sync.dma_start(qkvW[:, i, c, :], w[c * 128:(c + 1) * 128, :])
        woS = pWo.tile([128, NM, D], BF16)

        def load_wo():
            for mc in range(NM):
                nc.sync.dma_start(woS[:, mc, :], woT[mc * 128:(mc + 1) * 128, :])

        # ---- resident activations ----
        QT = pQKV.tile([128, NM, S], BF16, tag="qt")      # [pair dims, S]
        KT = pQKV.tile([128, NM, S], BF16, tag="kt")      # packed pair khT
        # col 64 = ones (softmax denominator rides in the AV stationary)
        VH = pQKV.tile([128, NSCH, 8, VW], BF16, tag="vh")
        nc.vector.memset(VH[:], 0.0)
        nc.vector.memset(VH[:, :, :, 64:65], 1.0)
        outT = pBig.tile([128, NM, S], BF16, tag="outt")  # [pair dims, S]

        if 'fake_p1' in knobs:  # timing experiments: satisfy deps cheaply
            knobs.add('no_p1')
            nc.vector.memset(QT[:], 0.001)
            nc.vector.memset(KT[:], 0.001)
            nc.vector.memset(VH[:], 1.0)

        # ---- phase 1 emitters (all blobs <= ~2-4us) ----
        def emit_qk_dma(i, T):
            xT = (xqT, xkT)[i]
            xts = [pX2.tile([128, 2, 512], BF16, tag="x", name=f"x{i}{T}{_c}")
                   for _c in range(KC)]
            for c in range(KC):
                for h2 in range(2):
                    nc.sync.dma_start(
                        xts[c][:, h2, :],
                        xT[c * 128:(c + 1) * 128,
                           T * 1024 + h2 * 512:T * 1024 + (h2 + 1) * 512])
            return xts

        def emit_qk_mm(i, T, xts, m):
            dst, bias = ((QT, bqS), (KT, bkS))[i]
            acc = psS.tile([128, 1024], F32, tag="sc", name=f"qk{i}{T}{m}")
            for h2 in range(2):
                for c in range(KC):
                    nc.tensor.matmul(
                        acc[:, h2 * 512:(h2 + 1) * 512],
                        qkvW[:, i, c, m * 128:(m + 1) * 128],
                        xts[c][:, h2, :],
                        start=(c == 0), stop=(c == KC - 1))
            nc.vector.tensor_scalar_add(
                dst[:, m, T * 1024:(T + 1) * 1024], acc[:], bias[:, m:m + 1])

        def emit_v_dma(t):
            xts = [pXV.tile([128, 512], BF16, tag="xv", name=f"xv{t}{_c}")
                   for _c in range(KC)]
            for c in range(KC):
                nc.sync.dma_start(
                    xts[c][:], xvT[c * 128:(c + 1) * 128, t * 512:(t + 1) * 512])
            return xts

        def emit_v_q(t, xts, u01, j):
            """One quarter of a V tile: 8 matmuls -> VH sch block 4t+2u+j."""
            sch = t * 4 + u01 * 2 + j
            acc = psS.tile([128, 512], F32, tag="sc", name=f"v{sch}")
            for c in range(KC):
                nc.tensor.matmul(
                    acc[:],
                    xts[c][:, (u01 * 2 + j) * 128:(u01 * 2 + j + 1) * 128],
                    qkvW[:, 2, c, :],
                    start=(c == 0), stop=(c == KC - 1))
            nc.vector.tensor_copy(
                VH[:, sch, :, 0:64], acc[:].rearrange("p (h d) -> p h d", h=8))

        # ---- phase 2 state ----
        acc2 = {}           # live AV accumulators for the current pass
        pending = []        # deferred normalize tails
        p3ready = []        # (sch, nt) halves whose qt column is flushed
        step_no = [0]
        prevq = []          # deep scores->av pipeline

        def emit_scores_exp(mh, qt, kb):
            et = pExp.tile([128, 1024], BF16, tag="expt",
                           name=f"et{mh}_{qt}_{kb}")
            sp = psS.tile([128, 1024], F32, tag="sc", name=f"sp{mh}_{qt}_{kb}")
            # two concurrent 64x128 row tiles: T0 = even head, T8 = odd head
            nc.tensor.matmul(
                sp[:, 0:512],
                KT[0:64, mh, kb * 128:(kb + 1) * 128],
                QT[0:64, mh, qt * 512:(qt + 1) * 512],
                start=True, stop=True)
            nc.tensor.matmul(
                sp[:, 512:1024],
                KT[64:128, mh, kb * 128:(kb + 1) * 128],
                QT[64:128, mh, qt * 512:(qt + 1) * 512],
                start=True, stop=True)
            if 'no_exp' not in knobs:
                nc.scalar.activation(
                    et[:], sp[:],
                    mybir.ActivationFunctionType.Exp, scale=0.125)
            return et

        def emit_av(mh, qt, kb, et):
            if 'no_av' in knobs:
                return
            if kb == 0:
                acc2[(mh, qt)] = [
                    psA.tile([128, 512], F32, tag="acc", name=f"av{mh}_{qt}{_h}")
                    for _h in range(2)]
            for hh in range(2):
                nc.tensor.matmul(
                    acc2[(mh, qt)][hh][:], VH[:, kb, 2 * mh + hh, :],
                    et[:, hh * 512:(hh + 1) * 512],
                    start=(kb == 0), stop=(kb == NSCH - 1))
            if kb == NSCH - 1 and 'no_norm' not in knobs:
                for hh in range(2):
                    # copy PSUM->SBUF fast so the accumulator bank frees
                    avs = pNrm.tile([65, 512], F32, tag="avs",
                                    name=f"avs{mh}_{qt}_{hh}")
                    nc.vector.tensor_copy(avs[:], acc2[(mh, qt)][hh][0:65, :])
                    recb = pRec.tile([1, 512], BF16, tag="recb",
                                     name=f"recb{mh}_{qt}_{hh}")
                    with nc.allow_low_precision("bf16 softmax reciprocal"):
                        nc.vector.reciprocal(recb[:], avs[64:65, :])
                    # due late enough that the 3.2us reciprocal has finished
                    pending.append((step_no[0] + (9 if hh == 0 else 11),
                                    hh * 64, mh, qt, avs, recb))
                del acc2[(mh, qt)]

        def flush_norm():
            # partition-broadcast 1/denom on GPSIMD, multiply, place in outT
            _, hp, mh, qt, avs, recb = pending.pop(0)
            bcb = pBC.tile([64, 512], BF16, tag="bcb", name=f"bc{mh}{qt}{hp}")
            nc.gpsimd.partition_broadcast(bcb[:], recb[:], channels=64)
            nrm = pNrm.tile([64, 512], BF16, tag="nrm", name=f"nrm{mh}{qt}{hp}")
            nc.vector.tensor_mul(nrm[:], avs[0:64, :], bcb[:])
            nc.sync.dma_start(
                outT[hp:hp + 64, mh, qt * 512:(qt + 1) * 512], nrm[:])
            if mh == NM - 1 and hp == 64:
                # last flush of this qt column: its p3 halves become usable a
                # few steps later (the flush's recip/broadcast/mul/DMA chain
                # takes ~5us wall; hosting earlier would block the PE queue)
                p3ready.extend((step_no[0] + 5, qt * 4 + j, nt)
                               for j in range(4) for nt in range(2))

        def emit_se_step(mh, qt, kb):
            # ready work first (deferred AV, due flushes): the PE queue is
            # in-order, so it must never sit behind a blocked scores matmul
            while len(prevq) >= AVLAG:
                emit_av(*prevq.pop(0))
            step_no[0] += 1
            while pending and step_no[0] >= pending[0][0]:
                flush_norm()
            et = emit_scores_exp(mh, qt, kb)
            prevq.append((mh, qt, kb, et))

        # ---- phase 3 emitter: one 512-wide half (4 matmuls) at a time ----
        def emit_p3_half(sch, nt):
            fp = psS.tile([128, 512], F32, tag="sc", name=f"fp{sch}_{nt}")
            for mc in range(NM):
                nc.tensor.matmul(
                    fp[:], outT[:, mc, sch * 128:(sch + 1) * 128],
                    woS[:, mc, nt * 512:(nt + 1) * 512],
                    start=(mc == 0), stop=(mc == NM - 1))
            of = pOutF.tile([128, 512], F32, tag="of", name=f"of{sch}_{nt}")
            nc.vector.tensor_copy(of[:], fp[:])
            nc.sync.dma_start(
                out[sch * 128:(sch + 1) * 128, nt * 512:(nt + 1) * 512], of[:])

        def drain_rest():
            # tail: remaining AVs are ready work; weave reserved p3 halves in
            while prevq:
                emit_av(*prevq.pop(0))
                if (p3ready and p3ready[0][0] <= step_no[0]
                        and len(prevq) % 3 == 0 and 'no_p3' not in knobs):
                    emit_p3_half(*p3ready.pop(0)[1:])
            while pending:
                flush_norm()
            if 'no_p3' not in knobs:
                while p3ready:
                    emit_p3_half(*p3ready.pop(0)[1:])

        # ---- emission sequence ----
        weave = {}
        if 'no_p1' not in knobs:
            load_w(0)
            xq0 = emit_qk_dma(0, 0)
            load_w(1)
            xk0 = emit_qk_dma(1, 0)
            load_w(2)
            xv0 = emit_v_dma(0)
            emit_qk_mm(0, 0, xq0, 0)
            emit_qk_mm(1, 0, xk0, 0)
            st = {}  # mutable closure state for woven dmas
            # ordering constraints: ALL readers of an x-tile set must be
            # emitted before the dma that reuses its pool slots (pX2: xk1
            # reuses xq0's slots, xq1 reuses xk0's; pXV rotates similarly).
            # AVLAG relaxes VH gating: VH block j is needed at step j+AVLAG.
            weave = {
                (0, 0): [
                    (0, lambda: emit_qk_mm(0, 0, xq0, 1)),
                    (1, lambda: emit_qk_mm(1, 0, xk0, 1)),
                    (2, lambda: st.__setitem__('xv1', emit_v_dma(1))),
                    (2, lambda: emit_qk_mm(0, 0, xq0, 2)),
                    (3, lambda: emit_qk_mm(1, 0, xk0, 2)),
                    (4, lambda: emit_qk_mm(0, 0, xq0, 3)),
                    (5, lambda: emit_qk_mm(1, 0, xk0, 3)),
                    (5, lambda: st.__setitem__('xk1', emit_qk_dma(1, 1))),
                    (6, lambda: emit_v_q(0, xv0, 0, 0)),
                    (7, lambda: emit_v_q(0, xv0, 0, 1)),
                    (8, lambda: emit_qk_mm(1, 1, st['xk1'], 0)),
                    (9, lambda: emit_v_q(0, xv0, 1, 0)),
                    (10, lambda: emit_v_q(0, xv0, 1, 1)),
                    (11, lambda: st.__setitem__('xv2', emit_v_dma(2))),
                    (12, lambda: emit_v_q(1, st['xv1'], 0, 0)),
                    (13, lambda: emit_v_q(1, st['xv1'], 0, 1)),
                    (14, lambda: emit_v_q(1, st['xv1'], 1, 0)),
                    (15, lambda: emit_v_q(1, st['xv1'], 1, 1)),
                ],
                (1, 0): [
                    (0, lambda: st.__setitem__('xv3', emit_v_dma(3))),
                    (0, lambda: emit_v_q(2, st['xv2'], 0, 0)),
                    (1, lambda: emit_v_q(2, st['xv2'], 0, 1)),
                    (2, lambda: emit_v_q(2, st['xv2'], 1, 0)),
                    (3, lambda: emit_v_q(2, st['xv2'], 1, 1)),
                    (4, lambda: emit_v_q(3, st['xv3'], 0, 0)),
                    (5, lambda: emit_v_q(3, st['xv3'], 0, 1)),
                    (6, lambda: emit_v_q(3, st['xv3'], 1, 0)),
                    (7, lambda: emit_v_q(3, st['xv3'], 1, 1)),
                    (8, lambda: emit_qk_mm(1, 1, st['xk1'], 1)),
                    (11, load_wo),
                ],
                (2, 0): [
                    (0, lambda: st.__setitem__('xq1', emit_qk_dma(0, 1))),
                    (2, lambda: emit_qk_mm(1, 1, st['xk1'], 2)),
                    (4, lambda: emit_qk_mm(0, 1, st['xq1'], 0)),
                    (6, lambda: emit_qk_mm(0, 1, st['xq1'], 1)),
                    (8, lambda: emit_qk_mm(0, 1, st['xq1'], 2)),
                    (10, lambda: emit_qk_mm(0, 1, st['xq1'], 3)),
                ],
                (3, 0): [
                    (2, lambda: emit_qk_mm(1, 1, st['xk1'], 3)),
                ],
            }
        elif 'no_p2' not in knobs:
            load_wo()

        if 'no_p2' not in knobs:
            for qt in range(4):
                for mh in range(NM):
                    todo = sorted(weave.get((mh, qt), []), key=lambda x: x[0])
                    for kb in range(NSCH):
                        while todo and todo[0][0] <= kb:
                            todo.pop(0)[1]()
                        # host ready p3 halves away from pass boundaries;
                        # none in the last pass (reserved for the tail)
                        if (kb in (2, 5, 8, 11) and p3ready
                                and p3ready[0][0] <= step_no[0]
                                and 'no_p3' not in knobs):
                            emit_p3_half(*p3ready.pop(0)[1:])
                        emit_se_step(mh, qt, kb)
            drain_rest()
        elif 'no_p1' not in knobs:
            # projections only (timing knob)
            emit_qk_mm(0, 0, xq0, 1)
            emit_qk_mm(0, 0, xq0, 2)
            emit_qk_mm(0, 0, xq0, 3)
            emit_qk_mm(1, 0, xk0, 1)
            emit_qk_mm(1, 0, xk0, 2)
            emit_qk_mm(1, 0, xk0, 3)
            for u01 in range(2):
                for j in range(2):
                    emit_v_q(0, xv0, u01, j)
            xq1 = emit_qk_dma(0, 1)
            xk1 = emit_qk_dma(1, 1)
            for m in range(NM):
                emit_qk_mm(0, 1, xq1, m)
                emit_qk_mm(1, 1, xk1, m)
            for t in range(1, 4):
                xv = emit_v_dma(t)
                for u01 in range(2):
                    for j in range(2):
                        emit_v_q(t, xv, u01, j)
            load_wo()
    return nc


def make_in_maps(q, k, v, Wq, bq, Wk, bk, Wv, bv, Wo, bo):
    """Shard + pre-transpose the full inputs into the 8 per-core maps."""
    q, k, v = (np.asarray(t, FP) for t in (q, k, v))
    Wq, bq, Wk, bk = (np.asarray(t, FP) for t in (Wq, bq, Wk, bk))
    Wv, bv, Wo, bo = (np.asarray(t, FP) for t in (Wv, bv, Wo, bo))
    maps = []
    for c in range(NCORES):
        b, g = c // 2, c % 2
        sl = slice(g * HD, (g + 1) * HD)
        maps.append({
            "xqT": np.ascontiguousarray(q[b].T).astype(BF),
            "xkT": np.ascontiguousarray(k[b].T).astype(BF),
            "xvT": np.ascontiguousarray(v[b].T).astype(BF),
            "wqT": np.ascontiguousarray(Wq[sl, :].T).astype(BF),
            "wkT": np.ascontiguousarray(Wk[sl, :].T).astype(BF),
            "wvT": np.ascontiguousarray(Wv[sl, :].T).astype(BF),
            "woT": np.ascontiguousarray(Wo[:, sl].T).astype(BF),
            "bq": np.ascontiguousarray(bq[sl].reshape(NM, 128).T),
            "bk": np.ascontiguousarray(bk[sl].reshape(NM, 128).T),
        })
    return maps


_CACHE = {}


def _get_program():
    if "nc" not in _CACHE:
        nc = bacc.Bacc("TRN2", target_bir_lowering=False, debug=False)
        build_core_program(nc)
        nc.compile()
        _CACHE["nc"] = nc
    return _CACHE["nc"]


def run(inputs, trace=False, **kw):
    """Run on the 8 NeuronCores; returns (full_output, BassKernelResults)."""
    nc = _get_program()
    in_maps = make_in_maps(**inputs)
    res = run_bass_kernel_spmd(
        nc, in_maps, core_ids=list(range(NCORES)), trace=trace, **kw)
    bv = np.asarray(inputs["bv"], FP)
    Wo = np.asarray(inputs["Wo"], FP)
    bo = np.asarray(inputs["bo"], FP)
    bias = bo + bv @ Wo.T
    full = np.empty((B, S, D), FP)
    for b in range(B):
        full[b] = (res.results[2 * b]["out"] + res.results[2 * b + 1]["out"]
                   + bias)
    return full, res


def kernel(**inputs) -> np.ndarray:
    # mask is all-ones by construction (spec fill: "ones") -> identity
    inputs.pop("mask", None)
    out, _ = run(inputs)
    return out
